# revision 7
# baseline (speedup 1.0000x reference)
"""Trainium2 Bass kernel v2 for nn_BaseMLP (per-node GNN message-passing MLP).

Reference computation (D=256 nodes, HID=64, P=2, BS=1024):
    xmask[b,j,t] = M[b,j,t] * adj[j,t] * x[b,j]
    h   = lrelu(einsum('tij,bjt->bti', W0, xmask) + b0)
    h   = lrelu(einsum('tij,btj->bti', W1, h) + b1)
    out =        einsum('tij,btj->bti', W2, h) + b2

Sharding: model-parallel over target-node dim t (32 t's per core).

Design (measured on HW, traced):
- Block-diagonal L1/L2 weights: each even/odd node pair becomes ONE
  128-contract matmul (PE busy 53.4 -> 40.5 us; matmul cost is streamed
  rhs columns regardless of lhsT width).
- All M-slab DMA triggers issued upfront on the SP HWDGE ring (all
  slabs fit in SBUF, no WAR waits); the first W0 chunk and slab 0 lead
  that ring; xt and the remaining weights/biases go via gpsimd SWDGE
  (own semaphores, idle Pool engine) so the scalar sequencer never
  issues DMA configs that would stall the ACT engine.
- Last two t-blocks stream as per-pair HALF slabs so the tail
  dependency chain (fold->L0->act->L1->act->L2->drain->out) runs on
  half-size units; their out-DMAs ride the then-idle SP ring.
- Output written f16 (host upcasts) to halve out-traffic.
- L2 PSUM->SBUF drains mostly on DVE (ACT is lrelu-bound); software-
  pipelined PE emission L0(tb) ... L2(tb-1) ... L1(tb) keeps the
  in-order PE queue from waiting on just-issued activations.
- The kernel is DMA-wire-bound: the ~19 MiB/core input stream at
  ~300-360 GB/s sets the floor; PE/ACT/DVE all fit underneath it.
"""

import numpy as np

D, HID, P, BS = 256, 64, 2, 1024
NCORES = 8
TLOC = D // NCORES          # 32 t's per core
JC = 2                      # j split into 2 chunks of 128 partitions
JP = 128
TBLK = 4                    # t's per M slab
NBT = TLOC // TBLK          # 8 t-blocks
NPAIR = TLOC // 2           # 16 pairs per core

TRACE = False
TRACE_CORES = None
LAST_RESULTS = None

# t-blocks optionally shipped as uint8 (round(M*255)) and expanded u8->f16
# on the ACT engine (scale=1/255 folded into the cast). Measured on-HW: the
# byte savings lose to the f16-slab delay it introduces, so default OFF.
import os as _os

U8TBS = tuple(
    int(t) for t in _os.environ.get("BASS_U8TBS", "").split(",") if t != ""
)
# drains (PSUM->SBUF) on ACT for these tbs, DVE for the rest
ACT_COPY_TBS = (1, 6)


# ---------------------------------------------------------------------------
# Toolchain workarounds (same as baseline): walrus accepts at most ONE sync
# wait per instruction; Tile emits several (worst on the tail drain).
# ---------------------------------------------------------------------------
def _install_patches():
    import bass_rust
    import concourse.tile as tile
    from concourse.vector_clock import ScopedClock

    if getattr(tile.TileContext, "_drain_patch_installed", False):
        return

    def _patched_drain_and_barrier(self, tick_clock, wait_clock):
        probe = self.nc.sync.nop()
        wait_clock.add_sem_waits(
            probe.ins, ScopedClock({None: tick_clock.global_clock})
        )
        si = probe.ins.sync_info
        waits = list(si.on_wait) if si is not None else []
        if len(waits) > 1:
            probe.ins.sync_info = bass_rust.SyncInfo(
                on_wait=[], on_update=list(si.on_update)
            )
            handles = {h.name: h for h in self.sems.allocated().values()}
            engs = [self.nc.sync, self.nc.vector, self.nc.scalar,
                    self.nc.gpsimd, self.nc.tensor]
            for i, w in enumerate(waits):
                engs[i % len(engs)].wait_ge(handles[w.ant_name], w.wait_value)
        drain_inst = self.nc.sync.drain()
        wait_clock.add_sem_waits(
            drain_inst.ins, ScopedClock({None: tick_clock.global_clock})
        )
        dsi = drain_inst.ins.sync_info
        if dsi is not None and len(dsi.on_wait) > 1:
            drain_inst.ins.sync_info = bass_rust.SyncInfo(
                on_wait=[], on_update=list(dsi.on_update)
            )
        self.nc.all_engine_barrier()
        assert self.sems is not None
        popped = self.nc._tile_sem_poison_stack.pop()
        assert popped is self._sem_poison
        self.nc.clear_and_free_semaphores(list(self.sems.allocated().values()))
        self.nc.all_engine_barrier()

    tile.TileContext._drain_and_barrier = _patched_drain_and_barrier
    tile.TileContext._drain_patch_installed = True


def _split_multiwait_instructions(nc):
    """Move extra sync waits onto single-wait NoOps inserted just before,
    on the same engine — ordering semantics preserved."""
    import bass_rust

    k = 0
    for fn in nc.m.functions:
        for bb in fn.blocks:
            insts = bb.instructions
            out = []
            changed = False
            for inst in insts:
                si = inst.sync_info
                waits = list(si.on_wait) if si is not None else []
                if len(waits) > 1:
                    changed = True
                    for w in waits[:-1]:
                        nop = bass_rust.InstNoOp(
                            name=f"mwsplit_{k}", ins=[], outs=[]
                        )
                        k += 1
                        nop.engine = inst.engine
                        nop.sync_info = bass_rust.SyncInfo(
                            on_wait=[w], on_update=[]
                        )
                        out.append(nop)
                    inst.sync_info = bass_rust.SyncInfo(
                        on_wait=[waits[-1]], on_update=list(si.on_update)
                    )
                out.append(inst)
            if changed:
                bb.instructions = out


def _install_ntff_hook():
    import sys
    import types

    try:
        from antenv.axon_hooks import get_axon_ntff_profile_hook  # noqa: F401

        return True
    except ImportError:
        pass
    mod = types.ModuleType("antenv.axon_hooks")
    _hook = [None]
    mod.set_axon_ntff_profile_hook = lambda h: _hook.__setitem__(0, h)
    mod.get_axon_ntff_profile_hook = lambda: _hook[0]
    sys.modules["antenv.axon_hooks"] = mod
    import antenv

    antenv.axon_hooks = mod
    try:
        from trn_agent_boot.trn_boot import _ntff_profile_via_ctypes

        mod.set_axon_ntff_profile_hook(
            _ntff_profile_via_ctypes("/opt/axon/libaxon_pjrt.so")
        )
        return True
    except Exception:
        return False


# ---------------------------------------------------------------------------
# Device program
# ---------------------------------------------------------------------------
_PROGRAM = {}


def _build_program(zero_b2: bool):
    import concourse.bass as bass
    import concourse.mybir as mybir
    import concourse.tile as tile
    from concourse.alu_op_type import AluOpType

    _install_patches()

    f32 = mybir.dt.float32
    f16 = mybir.dt.float16

    u8 = mybir.dt.uint8
    NU8 = max(1, len(U8TBS))

    nc = bass.Bass()
    mp = nc.dram_tensor("mp", [JC, NBT, JP, TBLK, BS], f16, kind="ExternalInput")
    mp8 = nc.dram_tensor("mp8", [JC, NU8, JP, TBLK, BS], u8, kind="ExternalInput")
    w0 = nc.dram_tensor("w0", [NBT, JP, JC, TBLK, HID], f16, kind="ExternalInput")
    w1 = nc.dram_tensor("w1", [JP, NPAIR, JP], f16, kind="ExternalInput")
    w2 = nc.dram_tensor("w2", [JP, NPAIR, 4], f16, kind="ExternalInput")
    xt = nc.dram_tensor("xt", [JC, JP, BS], f16, kind="ExternalInput")
    b0 = nc.dram_tensor("b0", [JP, NPAIR], f32, kind="ExternalInput")
    b1 = nc.dram_tensor("b1", [JP, NPAIR], f32, kind="ExternalInput")
    b2 = nc.dram_tensor("b2", [JP, NBT], f32, kind="ExternalInput")
    out = nc.dram_tensor("out", [NBT, TBLK, P, BS], f16, kind="ExternalOutput")

    Lrelu = mybir.ActivationFunctionType.Lrelu
    Copy = mybir.ActivationFunctionType.Copy
    NS = [slice(0, 512), slice(512, 1024)]

    with tile.TileContext(nc) as tc:
        with (
            tc.tile_pool(name="consts", bufs=1) as consts,
            tc.tile_pool(name="mslab", bufs=12) as mpool,
            tc.tile_pool(name="mhalf", bufs=8) as mhpool,
            tc.tile_pool(name="m8", bufs=max(1, 2 * len(U8TBS))) as m8pool,
            tc.tile_pool(name="htiles", bufs=6) as hpool,
            tc.tile_pool(name="otiles", bufs=3) as opool,
            tc.tile_pool(name="ps01", bufs=3, space="PSUM") as ps01pool,
            tc.tile_pool(name="ps2", bufs=1, space="PSUM") as ps2pool,
        ):
            # SP ring carries the hot-path loads in first-needed order:
            # xt, W0[tb0], then the full M-slab stream. Everything else
            # goes via gpsimd SWDGE (own semaphores, idle Pool engine) so
            # the scalar sequencer never issues a single DMA config and
            # the ACT engine can start immediately.
            # xt via SWDGE (desc-gen starts right after the preamble, the
            # transfer overlaps slab0) so the sync ring leads with the
            # first W0 chunk + M slab the PE needs first
            xt_sb = []
            for jc in range(JC):
                t_ = consts.tile([JP, BS], f16, name=f"xt{jc}")
                nc.gpsimd.dma_start(out=t_[:], in_=xt[jc, :, :])
                xt_sb.append(t_)
            w0_sb = consts.tile([JP, NBT, JC, TBLK, HID], f16)
            nc.sync.dma_start(out=w0_sb[:, 0], in_=w0[0])
            # M stream: full slabs for tb 0..5; the last two t-blocks
            # arrive as per-pair HALF slabs so the tail dependency chain
            # operates on half-size units.
            HTB = NBT - 2  # first half-slab t-block
            u8idx = {tb: k for k, tb in enumerate(U8TBS)}
            mts = [[None] * JC for _ in range(HTB)]
            mu8s = {}      # u8 staging tiles per tb

            def slab_dma(tb):
                for jc in range(JC):
                    mt = mpool.tile([JP, TBLK, BS], f16, tag="mslab")
                    if tb in u8idx:
                        m8 = m8pool.tile([JP, TBLK, BS], u8, tag="m8")
                        nc.sync.dma_start(out=m8[:], in_=mp8[jc, u8idx[tb]])
                        mu8s.setdefault(tb, []).append(m8)
                    else:
                        nc.sync.dma_start(out=mt[:], in_=mp[jc, tb])
                    mts[tb][jc] = mt

            # stream order: slab0 first (ramp), then the small u8 blocks
            # (cast during the ACT ramp-idle window), then the rest
            slab_dma(0)
            for tb in U8TBS:
                slab_dma(tb)
            for tb in range(1, HTB):
                if tb not in u8idx:
                    slab_dma(tb)
                if tb == 3:
                    # late W0 chunks ride the sync ring mid-stream: the
                    # ring is in-order, so they land ahead of slab4's
                    # compute while keeping the early wire pure slabs
                    for k in range(4, NBT):
                        nc.sync.dma_start(out=w0_sb[:, k], in_=w0[k])
            mts_h = {}     # (tb, pr)[jc] half tiles, arrival order
            for tb in range(HTB, NBT):
                for pr in range(2):
                    row = []
                    for jc in range(JC):
                        mt = mhpool.tile([JP, 2, BS], f16, tag="mhalf")
                        nc.sync.dma_start(
                            out=mt[:], in_=mp[jc, tb][:, 2 * pr : 2 * pr + 2]
                        )
                        row.append(mt)
                    mts_h[(tb, pr)] = row

            # gpsimd SWDGE: remaining weights/biases, earliest-needed first
            b0_sb = consts.tile([JP, NPAIR], f32)
            nc.gpsimd.dma_start(out=b0_sb[:], in_=b0[:, :])
            w1_sb = consts.tile([JP, NPAIR, JP], f16)
            nc.gpsimd.dma_start(out=w1_sb[:], in_=w1[:, :, :])
            b1_sb = consts.tile([JP, NPAIR], f32)
            nc.gpsimd.dma_start(out=b1_sb[:], in_=b1[:, :])
            nc.gpsimd.dma_start(out=w0_sb[:, 1], in_=w0[1])
            w2_sb = consts.tile([JP, NPAIR, 4], f16)
            nc.gpsimd.dma_start(out=w2_sb[:], in_=w2[:, :, :])
            for tb in range(2, 4):
                nc.gpsimd.dma_start(out=w0_sb[:, tb], in_=w0[tb])
            if not zero_b2:
                b2_sb = consts.tile([JP, NBT], f32)
                nc.gpsimd.dma_start(out=b2_sb[:], in_=b2[:, :])

            def cast_u8(tb):
                # expand u8 staging -> f16 compute tile on ACT (ramp window)
                for jc in range(JC):
                    nc.scalar.activation(
                        mts[tb][jc][:], mu8s[tb][jc][:], Copy,
                        bias=0.0, scale=1.0 / 255.0,
                    )

            def fold(tb):
                # slab[j, t, b] *= x^T[j, b]  (in place, DVE 2x mode)
                if tb < HTB:
                    for jc in range(JC):
                        nc.vector.tensor_tensor(
                            mts[tb][jc][:],
                            mts[tb][jc][:],
                            xt_sb[jc][:].unsqueeze(1).broadcast_to(
                                (JP, TBLK, BS)
                            ),
                            op=AluOpType.mult,
                        )
                else:
                    for pr in range(2):
                        for jc in range(JC):
                            mt = mts_h[(tb, pr)][jc]
                            nc.vector.tensor_tensor(
                                mt[:],
                                mt[:],
                                xt_sb[jc][:].unsqueeze(1).broadcast_to(
                                    (JP, 2, BS)
                                ),
                                op=AluOpType.mult,
                            )

            def pair_rows(tb, pr):
                # (tiles_by_jc, even_row, odd_row) for pair pr of tb
                if tb < HTB:
                    return mts[tb], 2 * pr, 2 * pr + 1
                return mts_h[(tb, pr)], 0, 1

            def l0(tb, pr):
                # pair pr of tb: even t -> ps0[0:64], odd t -> ps0[64:128]
                ps_ = ps01pool.tile([JP, BS], f32, tag="ps01")
                tiles, re, ro = pair_rows(tb, pr)
                for ns in NS:
                    for jc in range(JC):
                        nc.tensor.matmul(
                            ps_[0:HID, ns],
                            w0_sb[:, tb, jc, 2 * pr, :],
                            tiles[jc][:, re, ns],
                            start=(jc == 0),
                            stop=(jc == JC - 1),
                        )
                    for jc in range(JC):
                        nc.tensor.matmul(
                            ps_[HID:JP, ns],
                            w0_sb[:, tb, jc, 2 * pr + 1, :],
                            tiles[jc][:, ro, ns],
                            start=(jc == 0),
                            stop=(jc == JC - 1),
                        )
                return ps_

            def act_h1(tb, pr, ps_):
                p = tb * 2 + pr
                h1 = hpool.tile([JP, BS], f16, tag="h1")
                nc.scalar.activation(
                    h1[:], ps_[:], Lrelu,
                    bias=b0_sb[:, p : p + 1], scale=1.0, alpha=0.01,
                )
                return h1

            def l1(tb, pr, h1):
                # block-diag pair matmul: ONE 128-contract matmul per ns
                p = tb * 2 + pr
                ps_ = ps01pool.tile([JP, BS], f32, tag="ps01")
                for ns in NS:
                    nc.tensor.matmul(
                        ps_[:, ns], w1_sb[:, p, :], h1[:, ns],
                        start=True, stop=True,
                    )
                return ps_

            def act_h2(tb, pr, ps_):
                p = tb * 2 + pr
                h2 = hpool.tile([JP, BS], f16, tag="h2")
                nc.scalar.activation(
                    h2[:], ps_[:], Lrelu,
                    bias=b1_sb[:, p : p + 1], scale=1.0, alpha=0.01,
                )
                return h2

            def l2(tb, h2s):
                # both pairs into one PSUM tile: pair pr at partitions
                # 32*pr .. 32*pr+3 via tile_position col offset
                ps_ = ps2pool.tile([JP, BS], f32, tag="ps2")
                for pr in range(2):
                    p = tb * 2 + pr
                    for ns in NS:
                        nc.tensor.matmul(
                            ps_[32 * pr : 32 * pr + 4, ns],
                            w2_sb[:, p, :],
                            h2s[pr][:, ns],
                            start=True, stop=True,
                            tile_position=(0, 32 * pr),
                        )
                return ps_

            def drain(tb, ps_):
                # PSUM -> SBUF f16; 3 of 8 tbs on DVE, rest on ACT
                osb = opool.tile([JP, BS], f16, tag="osb")
                if not zero_b2:
                    nc.vector.tensor_scalar_add(
                        osb[:], ps_[:], b2_sb[:, tb : tb + 1]
                    )
                elif tb in ACT_COPY_TBS:
                    nc.scalar.activation(
                        osb[:], ps_[:], Copy, bias=0.0, scale=1.0
                    )
                else:
                    nc.vector.tensor_copy(osb[:], ps_[:])
                eng = nc.sync if tb >= HTB else nc.gpsimd
                for pr in range(2):
                    eng.dma_start(
                        out=out[tb, 2 * pr : 2 * pr + 2],
                        in_=osb[32 * pr : 32 * pr + 4],
                    )

            # u8 expansions first in ACT order — they run in the ramp
            # window while ACT waits for the first L0 results
            for tb in U8TBS:
                cast_u8(tb)

            # software-pipelined emission:
            #   fold(tb) ; L0(tb) ; [L2(tb-1); drain(tb-1)] ; L1+acts(tb)
            prev = None  # (tb, h2s) pending L2
            for tb in range(NBT):
                fold(tb)
                ps0s = [l0(tb, 0), l0(tb, 1)]
                h1s = [act_h1(tb, 0, ps0s[0]), act_h1(tb, 1, ps0s[1])]
                if prev is not None:
                    ps2 = l2(prev[0], prev[1])
                    drain(prev[0], ps2)
                ps1s = [l1(tb, 0, h1s[0]), l1(tb, 1, h1s[1])]
                h2s = [act_h2(tb, 0, ps1s[0]), act_h2(tb, 1, ps1s[1])]
                prev = (tb, h2s)
            ps2 = l2(prev[0], prev[1])
            drain(prev[0], ps2)

    _split_multiwait_instructions(nc)
    return nc


def _get_program(zero_b2: bool):
    if zero_b2 not in _PROGRAM:
        _PROGRAM[zero_b2] = _build_program(zero_b2)
    return _PROGRAM[zero_b2]


# ---------------------------------------------------------------------------
# Host wrapper
# ---------------------------------------------------------------------------
def kernel(x, M, adj, W0, b0, W1, b1, W2, b2):
    global LAST_RESULTS
    from concourse import bass_utils

    x = np.asarray(x, np.float32)
    M = np.asarray(M, np.float32)
    adj = np.asarray(adj, np.float32)
    W0 = np.asarray(W0, np.float32)
    b0 = np.asarray(b0, np.float32)
    W1 = np.asarray(W1, np.float32)
    b1 = np.asarray(b1, np.float32)
    W2 = np.asarray(W2, np.float32)
    b2 = np.asarray(b2, np.float32)

    xt_full = np.ascontiguousarray(x.T.astype(np.float16)).reshape(JC, JP, BS)

    def pack_pairs(a):
        # a: (TLOC, HID, ...) -> (128, NPAIR, ...): rows 0:64 even t,
        # rows 64:128 odd t
        ev, od = a[0::2], a[1::2]
        return np.concatenate([ev, od], axis=1).transpose(
            (1, 0) + tuple(range(2, a.ndim))
        )

    in_maps = []
    for c in range(NCORES):
        tsl = slice(c * TLOC, (c + 1) * TLOC)
        mre = (
            M[:, :, tsl]
            .transpose(1, 2, 0)
            .reshape(JC, JP, NBT, TBLK, BS)
            .transpose(0, 2, 1, 3, 4)
        )  # (JC, NBT, JP, TBLK, BS)
        mpc = np.ascontiguousarray(mre).astype(np.float16)
        if U8TBS:
            mp8c = np.ascontiguousarray(
                np.rint(mre[:, list(U8TBS)] * 255.0)
            ).astype(np.uint8)
        else:
            mp8c = np.zeros((JC, 1, JP, TBLK, BS), np.uint8)
        # fold adj into W0: W0eff[t,i,j] = W0[t,i,j] * adj[j,t]
        w0eff = W0[tsl] * adj.T[tsl][:, None, :]          # (TLOC, HID, D)
        w0l = np.ascontiguousarray(
            w0eff.transpose(2, 0, 1)                       # (D, TLOC, HID)
            .reshape(JC, JP, NBT, TBLK, HID)
            .transpose(2, 1, 0, 3, 4)                      # (NBT, JP, JC, TBLK, HID)
        ).astype(np.float16)
        # block-diag pair packing for L1/L2
        w1t = W1[tsl].transpose(0, 2, 1)                   # (TLOC, j_in, i_out)
        w2t = W2[tsl].transpose(0, 2, 1)                   # (TLOC, j_in, p_out)
        w1l = np.zeros((JP, NPAIR, JP), np.float16)
        w2l = np.zeros((JP, NPAIR, 4), np.float16)
        for pp in range(NPAIR):
            w1l[0:HID, pp, 0:HID] = w1t[2 * pp]
            w1l[HID:JP, pp, HID:JP] = w1t[2 * pp + 1]
            w2l[0:HID, pp, 0:P] = w2t[2 * pp]
            w2l[HID:JP, pp, P : 2 * P] = w2t[2 * pp + 1]
        b0t = b0[tsl]
        b1t = b1[tsl]
        b0l = np.ascontiguousarray(pack_pairs(b0t[:, :, None])[:, :, 0]).astype(
            np.float32
        )
        b1l = np.ascontiguousarray(pack_pairs(b1t[:, :, None])[:, :, 0]).astype(
            np.float32
        )
        b2t = b2[tsl]                                      # (TLOC, P)
        b2l = np.zeros((JP, NBT), np.float32)
        for t in range(TLOC):
            tb, r = divmod(t, TBLK)
            pr, o = divmod(r, 2)
            for pv in range(P):
                b2l[32 * pr + 2 * o + pv, tb] = b2t[t, pv]
        in_maps.append(
            {
                "mp": mpc,
                "mp8": mp8c,
                "w0": w0l,
                "w1": w1l,
                "w2": w2l,
                "xt": xt_full,
                "b0": b0l,
                "b1": b1l,
                "b2": b2l,
            }
        )

    nc = _get_program(zero_b2=not np.any(b2))
    kw = {}
    if TRACE:
        _install_ntff_hook()
        kw["trace"] = True
        if TRACE_CORES is not None:
            kw["trace_cores"] = TRACE_CORES
    res = bass_utils.run_bass_kernel_spmd(
        nc, in_maps, core_ids=list(range(NCORES)), **kw
    )
    LAST_RESULTS = res

    out = np.empty((BS, D, P), np.float32)
    for c in range(NCORES):
        tsl = slice(c * TLOC, (c + 1) * TLOC)
        r = res.results[c]["out"].astype(np.float32).reshape(TLOC, P, BS)
        out[:, tsl, :] = r.transpose(2, 0, 1)
    return out
